# revision 37
# baseline (speedup 1.0000x reference)
"""Multi-head attention (B=2, S=2048, H=1024, 16 heads) on 8 TRN2 NeuronCores.

Sharding: tensor-parallel heads x data-parallel batch. Core c -> batch c//4,
head group c%4 (4 heads each). Megatron-style partial out-projections summed
on the host.

V4 design:
- Natural V computed directly: per 128-token chunk, stationary = xT chunk
  [128 hid, 128 tok], moving = Wv [128 hid, 256] accumulated over hidden
  chunks -> [128 tok, 256] PSUM, then one strided DVE copy into the
  per-(t, h) V strips. No V^T staging, no transposes.
- Q^T/K^T per-head strips duplicated across both partition bands (the
  concurrent row-tiled scores pair must read both bands at the SAME free
  offset of the same tile -- split tiles raced on real hardware).
- Phase A order Q(ci0), K(ci0), V, Q(ci1), K(ci1): scores for head-pair 0
  start ~16us in; V strips are produced t-ascending just ahead of the first
  ctx consumption.
- j-outer attention pipeline per 512-query block, heads ordered (1,3,0,2)
  so the last division before each out-projection is an even head (no
  cross-band DMA hop on the critical path). ctx accumulates alternately
  into two PSUM banks (A: even chunks, B: odd chunks) to dodge same-bank
  accumulation turnaround; division does craw = A + B which frees both.
- Softmax denominator via the ones-column row-64 trick; reciprocal on a
  [128, 4] DMA-gathered layout.
- exp split: ACT 5/8 of tiles (real exp), DVE 3/8 via Schraudolph fast-exp
  (int16 bits = s*a + b bitcast as bf16, bias tuned so the mean
  multiplicative error matches exp for consistent mixing).
- bk dropped: it shifts every score of a query equally and softmax cancels
  it. bv/bo are folded into a host-side additive constant; bq applied
  on-device.
"""

import ml_dtypes
import numpy as np

import concourse.bacc as bacc
import concourse.mybir as mybir
import concourse.tile as tile
from concourse.bass_utils import run_bass_kernel_spmd

NCORES = 8
B, S, HID = 2, 2048, 1024
NH, HD = 16, 64
HPC = 4            # heads per core
QC = HPC * HD      # 256 local projection cols per core
HC = HID // 128    # 8 hidden chunks
TC = S // 128      # 16 token chunks
TB = S // 512      # 4 query blocks
NCP = TC // 2      # 8 chunk-pairs per head

F32 = mybir.dt.float32
BF16 = mybir.dt.bfloat16
FP16 = mybir.dt.float16
I16 = mybir.dt.int16
EXP = mybir.ActivationFunctionType.Exp
MULT = mybir.AluOpType.mult
ADD = mybir.AluOpType.add

SCH_A = 128.0 / float(np.log(2.0))
SCH_B = 16256.0 - 7.0
SCH_CPS = (3, 6)
OP_DELAY = 12
HEAD_ORDER = (1, 3, 0, 2)


def build_nc():
    nc = bacc.Bacc("TRN2", target_bir_lowering=False, debug=False,
                   num_devices=NCORES)
    xT = nc.declare_dram_parameter("xT", [HID, S], FP16, isOutput=False)
    wq = nc.declare_dram_parameter("wq", [HID, QC], FP16, isOutput=False)
    wk = nc.declare_dram_parameter("wk", [HID, QC], FP16, isOutput=False)
    wv = nc.declare_dram_parameter("wv", [HID, QC], FP16, isOutput=False)
    wo = nc.declare_dram_parameter("wo", [QC, HID], BF16, isOutput=False)
    bq = nc.declare_dram_parameter("bq", [QC], F32, isOutput=False)
    out = nc.declare_dram_parameter("out", [S, HID], BF16, isOutput=True)

    with tile.TileContext(nc) as tc:
        with (
            tc.tile_pool(name="const", bufs=1) as constp,
            tc.tile_pool(name="qkv", bufs=1) as qkvp,
        ):
            wo_sb = constp.tile([128, 2 * HID], BF16)
            bq_sb = constp.tile([128, 2], F32)
            # per-head strips, duplicated across both partition bands
            qt2 = qkvp.tile([128, HPC * S], FP16)
            kt2 = qkvp.tile([128, HPC * S], FP16)
            # natural V strips: strip (t*HPC + h) at col offset *128,
            # V in cols 0:64, ones in col 64
            v_sb = qkvp.tile([128, TC * HPC * 128], BF16)
            ctxf_sb = qkvp.tile([128, 2 * S], BF16)
            xT_sb = qkvp.tile([128, HC * S], FP16)
            wv_sb = qkvp.tile([128, HC * QC], FP16)

            ones_ap = v_sb[:, :].rearrange("p (s e) -> p s e", e=128)[:, :, HD:HD + 1]
            nc.vector.memset(ones_ap, 1.0)

            # ---- phase A: projections --------------------------------
            with (
                tc.tile_pool(name="xw", bufs=1) as xwp,
                tc.tile_pool(name="ps1", bufs=4, space="PSUM") as ps1,
            ):
                wq_sb = xwp.tile([128, HC * QC], FP16)
                wk_sb = xwp.tile([128, HC * QC], FP16)

                xt_dmas = {}
                for w_sb_, w_ in ((wq_sb, wq), (wk_sb, wk)):
                    for hc in range(HC):
                        r = slice(hc * 128, (hc + 1) * 128)
                        nc.scalar.dma_start(
                            w_sb_[:, hc * QC:(hc + 1) * QC], w_[r, :])
                for hc in range(HC):
                    r = slice(hc * 128, (hc + 1) * 128)
                    eng = nc.sync if hc % 2 == 0 else nc.scalar
                    if hc <= 1:
                        for j in range(TB):
                            xt_dmas[hc] = eng.dma_start(
                                xT_sb[:, hc * S + j * 512:
                                      hc * S + (j + 1) * 512],
                                xT[r, j * 512:(j + 1) * 512])
                    else:
                        xt_dmas[hc] = eng.dma_start(
                            xT_sb[:, hc * S:(hc + 1) * S], xT[r, :])
                for hc in range(HC):
                    r = slice(hc * 128, (hc + 1) * 128)
                    nc.scalar.dma_start(wv_sb[:, hc * QC:(hc + 1) * QC],
                                        wv[r, :])
                for ci in range(2):
                    nc.sync.dma_start(bq_sb[:, ci:ci + 1],
                                      bq[ci * 128:(ci + 1) * 128])

                qk_mms = {}

                def emit_qk(name, w_sb, dst, ci, with_bias):
                    hA, hB = 2 * ci, 2 * ci + 1
                    for jh in range(2):
                        ps = ps1.tile([128, 1024], F32, tag="ps1")
                        for hc in range(HC):
                            for jj in range(2):
                                j = 2 * jh + jj
                                qk_mms[(name, ci, hc, j)] = nc.tensor.matmul(
                                    ps[:, jj * 512:(jj + 1) * 512],
                                    w_sb[:, hc * QC + ci * 128:
                                         hc * QC + ci * 128 + 128],
                                    xT_sb[:, hc * S + j * 512:
                                          hc * S + j * 512 + 512],
                                    start=(hc == 0), stop=(hc == HC - 1))
                        cols = slice(2 * jh * 512, 2 * (jh + 1) * 512)
                        sA = slice(hA * S + 2 * jh * 512,
                                   hA * S + 2 * (jh + 1) * 512)
                        sB = slice(hB * S + 2 * jh * 512,
                                   hB * S + 2 * (jh + 1) * 512)
                        if with_bias:
                            nc.vector.tensor_scalar_add(
                                dst[0:64, sA], ps[0:64, :],
                                bq_sb[0:64, ci:ci + 1])
                            nc.scalar.activation(
                                dst[64:128, sB], ps[64:128, :],
                                mybir.ActivationFunctionType.Identity,
                                bias=bq_sb[64:128, ci:ci + 1])
                        else:
                            nc.vector.tensor_copy(dst[0:64, sA], ps[0:64, :])
                            nc.scalar.copy(dst[64:128, sB], ps[64:128, :])
                    sA = slice(hA * S, (hA + 1) * S)
                    sB = slice(hB * S, (hB + 1) * S)
                    nc.gpsimd.dma_start(dst[64:128, sA], dst[0:64, sA])
                    nc.gpsimd.dma_start(dst[0:64, sB], dst[64:128, sB])

                emit_qk("q", wq_sb, qt2, 0, True)
                emit_qk("k", wk_sb, kt2, 0, False)
                emit_qk("q", wq_sb, qt2, 1, True)
                emit_qk("k", wk_sb, kt2, 1, False)

                for hc in range(2, HC):
                    tile.add_dep_helper(xt_dmas[hc].ins,
                                        qk_mms[("q", 0, hc - 2, 3)].ins,
                                        reason="pace xT load")
                for ci in range(2):
                    d = nc.scalar.dma_start(
                        wo_sb[:, ci * HID:(ci + 1) * HID],
                        wo[ci * 128:(ci + 1) * 128, :])
                    tile.add_dep_helper(d.ins, qk_mms[("k", 0, 5 + ci, 1)].ins,
                                        reason="pace wo load")

            # ---- phase B: attention, j-outer pipeline --------------------
            with (
                tc.tile_pool(name="bigps", bufs=3, space="PSUM") as bigps,
                tc.tile_pool(name="ctps", bufs=2, space="PSUM") as ctps,
                tc.tile_pool(name="probs", bufs=6) as probsp,
                tc.tile_pool(name="div", bufs=2) as divp,
                tc.tile_pool(name="ostg", bufs=3) as ostg,
            ):
                stages = [(j, h, cp) for j in range(TB) for h in HEAD_ORDER
                          for cp in range(NCP)]
                probs_tiles = {}
                ctx_tiles = {}

                def emit_v_chunk(t):
                    vt = bigps.tile([128, 1024], F32, tag="big")
                    for hc in range(HC):
                        nc.tensor.matmul(
                            vt[:, 0:256],
                            xT_sb[:, hc * S + t * 128:
                                  hc * S + (t + 1) * 128],
                            wv_sb[:, hc * QC:(hc + 1) * QC],
                            start=(hc == 0), stop=(hc == HC - 1))
                    vsrc = vt[:, 0:256].rearrange("p (h e) -> p h e", e=HD)
                    dst = v_sb[:, t * 512:(t + 1) * 512].rearrange(
                        "p (h e) -> p h e", e=128)[:, :, 0:HD]
                    nc.vector.tensor_copy(dst, vsrc)

                def emit_scores(j, h, cp):
                    hS = h * S
                    c0, c1 = 2 * cp, 2 * cp + 1
                    sp = bigps.tile([128, 1024], F32, tag="big")
                    probs_c = probsp.tile([128, 1024], BF16, tag="probs",
                                          name=f"probs_j{j}h{h}cp{cp}")
                    probs_tiles[(j, h, cp)] = probs_c
                    nc.tensor.matmul(
                        sp[:, 0:512],
                        kt2[0:64, hS + c0 * 128:hS + (c0 + 1) * 128],
                        qt2[0:64, hS + j * 512:hS + (j + 1) * 512],
                        start=True, stop=True)
                    nc.tensor.matmul(
                        sp[:, 512:1024],
                        kt2[64:128, hS + c1 * 128:hS + (c1 + 1) * 128],
                        qt2[64:128, hS + j * 512:hS + (j + 1) * 512],
                        start=True, stop=True)
                    if cp in SCH_CPS:
                        nc.vector.tensor_scalar(
                            probs_c[:, :].bitcast(I16), sp[:, :],
                            SCH_A, SCH_B, op0=MULT, op1=ADD)
                    else:
                        nc.scalar.activation(probs_c[:, :], sp[:, :], EXP)

                def emit_ctx(j, h, cp):
                    if cp == 0:
                        ctx_tiles[(j, h)] = (
                            ctps.tile([65, 512], F32, tag="ctx",
                                      name=f"ctxA_j{j}h{h}"),
                            ctps.tile([65, 512], F32, tag="ctx",
                                      name=f"ctxB_j{j}h{h}"))
                    ctx_ab = ctx_tiles[(j, h)]
                    probs_c = probs_tiles.pop((j, h, cp))
                    for ck in range(2):
                        c = 2 * cp + ck
                        vbase = (c * HPC + h) * 128
                        nc.tensor.matmul(
                            ctx_ab[ck][0:65, :],
                            v_sb[:, vbase:vbase + 65],
                            probs_c[:, ck * 512:(ck + 1) * 512],
                            start=(cp == 0), stop=(cp == NCP - 1))

                def emit_division(j, h):
                    ci, half = h // 2, h % 2
                    ctxA, ctxB = ctx_tiles.pop((j, h))
                    crawA = divp.tile([65, 512], F32, tag="crawA")
                    nc.vector.tensor_copy(crawA[0:65, :], ctxA[0:65, :])
                    craw = divp.tile([65, 512], F32, tag="craw")
                    nc.vector.tensor_tensor(out=craw[0:65, :],
                                            in0=ctxB[0:65, :],
                                            in1=crawA[0:65, :], op=ADD)
                    denr = divp.tile([128, 4], F32, tag="denr")
                    nc.gpsimd.dma_start(denr[:, :], craw[64:65, :])
                    recr = divp.tile([128, 4], F32, tag="recr")
                    nc.vector.reciprocal(recr[:], denr[:])
                    rrow = divp.tile([1, 512], F32, tag="rrow")
                    nc.gpsimd.dma_start(rrow[:, :], recr[:, :])
                    Dt = divp.tile([64, 512], F32, tag="Dt")
                    nc.gpsimd.partition_broadcast(Dt[:, :], rrow[0:1, :])
                    dst_cols = slice(ci * S + j * 512, ci * S + (j + 1) * 512)
                    if half == 0:
                        nc.vector.tensor_tensor(
                            out=ctxf_sb[0:64, dst_cols],
                            in0=craw[0:64, :], in1=Dt[0:64, :], op=MULT)
                    else:
                        ctxd = divp.tile([64, 512], BF16, tag="ctxd")
                        nc.vector.tensor_tensor(
                            out=ctxd[0:64, :],
                            in0=craw[0:64, :], in1=Dt[0:64, :], op=MULT)
                        nc.scalar.dma_start(ctxf_sb[64:128, dst_cols],
                                            ctxd[0:64, :])

                def emit_outproj_t(t):
                    op = bigps.tile([128, 1024], F32, tag="big")
                    for ci in range(2):
                        for oc in range(2):
                            nc.tensor.matmul(
                                op[:, oc * 512:(oc + 1) * 512],
                                ctxf_sb[:, ci * S + t * 128:
                                        ci * S + t * 128 + 128],
                                wo_sb[:, ci * HID + oc * 512:
                                      ci * HID + oc * 512 + 512],
                                start=(ci == 0), stop=(ci == 1))
                    ot = ostg.tile([128, 1024], BF16, tag="ot")
                    nc.scalar.copy(ot[:, 0:512], op[:, 0:512])
                    nc.vector.tensor_copy(ot[:, 512:1024], op[:, 512:1024])
                    nc.sync.dma_start(out[t * 128:(t + 1) * 128, :], ot[:, :])

                LEAD = 3
                op_due = {}
                nsteps = len(stages) + LEAD + OP_DELAY + 4
                for i in range(nsteps):
                    if i < len(stages):
                        emit_scores(*stages[i])
                    if i < NCP:
                        emit_v_chunk(2 * i)
                        emit_v_chunk(2 * i + 1)
                    if LEAD <= i < len(stages) + LEAD:
                        j, h, cp = stages[i - LEAD]
                        emit_ctx(j, h, cp)
                        if cp == NCP - 1:
                            emit_division(j, h)
                            if h == HEAD_ORDER[-1]:
                                delay = OP_DELAY if j < TB - 1 else 1
                                for tt in range(4):
                                    op_due.setdefault(
                                        i + delay + tt, []).append(4 * j + tt)
                    for t in op_due.pop(i, ()):
                        emit_outproj_t(t)

    nc.compile()
    return nc


_NC = None


def _get_nc():
    global _NC
    if _NC is None:
        _NC = build_nc()
    return _NC


def make_in_maps(x, Wq, bq, Wk, bk, Wv, bv, Wo, bo):
    in_maps = []
    for core in range(NCORES):
        b, g = core // 4, core % 4
        sl = slice(g * QC, (g + 1) * QC)
        in_maps.append({
            "xT": np.ascontiguousarray(x[b].T).astype(np.float16),
            "wq": (np.ascontiguousarray(Wq[:, sl]) * 0.125).astype(np.float16),
            "wk": np.ascontiguousarray(Wk[:, sl]).astype(np.float16),
            "wv": np.ascontiguousarray(Wv[:, sl]).astype(np.float16),
            "wo": np.ascontiguousarray(Wo[sl, :]).astype(ml_dtypes.bfloat16),
            "bq": (np.asarray(bq[sl]) * 0.125).astype(np.float32),
        })
    return in_maps


def combine_outputs(core_outs, Wv_bias_term):
    full = np.empty((B, S, HID), np.float32)
    for b in range(B):
        acc = core_outs[4 * b].astype(np.float32).copy()
        for g in range(1, 4):
            acc += core_outs[4 * b + g]
        full[b] = acc + Wv_bias_term
    return full


def kernel(**inputs):
    x = np.asarray(inputs["x"], np.float32)
    Wq = np.asarray(inputs["Wq"], np.float32)
    bq = np.asarray(inputs["bq"], np.float32)
    Wk = np.asarray(inputs["Wk"], np.float32)
    bk = np.asarray(inputs["bk"], np.float32)
    Wv = np.asarray(inputs["Wv"], np.float32)
    bv = np.asarray(inputs["bv"], np.float32)
    Wo = np.asarray(inputs["Wo"], np.float32)
    bo = np.asarray(inputs["bo"], np.float32)

    nc = _get_nc()
    in_maps = make_in_maps(x, Wq, bq, Wk, bk, Wv, bv, Wo, bo)
    res = run_bass_kernel_spmd(nc, in_maps, core_ids=list(range(NCORES)))
    core_outs = [res.results[c]["out"] for c in range(NCORES)]
    bias_term = (bv @ Wo + bo).astype(np.float32)
    return combine_outputs(core_outs, bias_term)
